# revision 11
# baseline (speedup 1.0000x reference)
"""Trainium2 Bass kernel for nn_BaselineProt (embedding_lookup).

The reference computes, per drug-pair sample:
    multihot(drug) @ W0.T  ==  sum of W0 columns at the drug's (deduped)
    target proteins -- i.e. an embedding-table gather/sum, followed by a
    tiny MLP tower on each leg and a dot product between the two legs.

Structure (8 NeuronCores, data-parallel):
  Launch A: drugs sharded 500/core (padded to 512). Each core issues 4
      large dma_gathers (4096 rows of the bf16 W0T table each; dups
      remapped to a zero row to preserve `.set` multihot semantics) and
      tree-reduces each sub-batch into an E-table shard [512, 256].
  Host:     concatenates the 8 E shards + the 32 cell-line columns into
      one lookup table E_ext [4128, 256] (pure data movement).
  Launch B: batch sharded 1024 samples/core. Two transpose-mode gathers
      pull E[d0], E[d1], cell rows in feature-major, leg-blocked column
      layout; unit-stride DVE adds + ACT ReLU(+b0) form h0; two matmul
      layers (W1, W2) and a ones-matmul pair-dot produce [1024] outputs.
"""

import os

os.environ.setdefault("JAX_PLATFORMS", "")

import numpy as np
import ml_dtypes

import concourse.bacc as bacc
import concourse.mybir as mybir
from concourse.tile import TileContext
from concourse import library_config
from concourse.bass_utils import run_bass_kernel_spmd

# Problem constants (hardcoded per harness contract).
B = 8192            # samples
P = 19000           # proteins
C = 32              # cell lines
D = 4000            # drugs
T = 32              # targets per drug
F = 256             # first hidden dim
H1 = 128            # second hidden dim
H2 = 64             # output dim per tower

NCORES = 8
DRUGS_PER_CORE = D // NCORES          # 500
DRUGS_PAD = 512                       # per-core padded drug count
SAMPLES_PER_CORE = B // NCORES        # 1024
ZROW = P + C                          # zero row in the W0T table (19032)
TAB_ROWS = ZROW + 8                   # pad table rows to 19040
E_ROWS = NCORES * DRUGS_PAD           # 4096 rows of E
EXT_ROWS = E_ROWS + C                 # + 32 cell rows = 4128
NI_A = DRUGS_PAD * T                  # 16384 gather idxs per core, launch A
NI_B = 3 * SAMPLES_PER_CORE           # 3072 gather idxs per core, launch B
NSUB = 4                              # launch A sub-batches of 128 drugs
NWAVE = 2                             # launch B gather waves
SW = SAMPLES_PER_CORE // NWAVE        # 512 samples per wave
NQ = 4                                # SWDGE queues

_BF16 = mybir.dt.bfloat16
_F32 = mybir.dt.float32
_I16 = mybir.dt.int16

_cache = {}


def _wrap_idx(flat):
    """Flat gather order -> the [128, n/16] int16 SBUF layout dma_gather
    expects (idx i at partition i%16, slot i//16; replicated to all 8 Q7
    core slices)."""
    n = flat.shape[0]
    assert n % 16 == 0
    arr = flat.astype(np.int16).reshape(n // 16, 16).T.copy()
    return np.tile(arr, (8, 1))


def _build_kernel_a():
    # 64KB/partition descriptor carveout -> 4096-desc SWDGE ring per queue
    # (the 16KB default holds only 1024 descs; bigger gathers corrupt).
    nc = bacc.Bacc("TRN2", target_bir_lowering=True, num_swdge_queues=NQ,
                   dynamic_dma_scratch_size=65536)
    tab = nc.dram_tensor("tab", [TAB_ROWS, F], _BF16, kind="ExternalInput")
    idxs = nc.dram_tensor("idxs", [128, NI_A // 16], _I16, kind="ExternalInput")
    e_out = nc.dram_tensor("e_out", [DRUGS_PAD, F], _BF16, kind="ExternalOutput")

    NG_A = 8                                      # gathers (2 per sub-batch)
    NI_S = NI_A // NG_A                           # 2048 idxs per gather
    TH = T // 2                                   # 16 target slots per gather
    with TileContext(nc) as tc:
        # library first: its ~6us IRAM load overlaps the idx DMA
        nc.gpsimd.load_library(library_config.mlp)
        with (
            tc.tile_pool(name="idx", bufs=1) as ip,
            tc.tile_pool(name="g", bufs=1) as gp,
            tc.tile_pool(name="e", bufs=2) as ep,
        ):
            idx_t = ip.tile([128, NI_A // 16], _I16)
            nc.sync.dma_start(out=idx_t[:, :], in_=idxs[:, :])
            # gather g covers sub-batch g//2, target half g%2
            gs = []
            for g in range(NG_A):
                gt = gp.tile([128, TH, F], _BF16, tag=f"g{g}")
                nc.gpsimd.dma_gather(
                    gt[:, :, :],
                    tab[:],
                    idx_t[:, g * (NI_S // 16):(g + 1) * (NI_S // 16)],
                    NI_S, NI_S, F,
                    single_packet=False, queue_num=g % NQ,
                )
                gs.append(gt)
            # tree-reduce 16 slots per gather, then combine the two halves
            for g in range(NG_A):
                gt = gs[g]
                w = TH // 2
                while w >= 1:
                    nc.vector.tensor_tensor(
                        out=gt[:, 0:w, :],
                        in0=gt[:, 0:w, :],
                        in1=gt[:, w:2 * w, :],
                        op=mybir.AluOpType.add,
                    )
                    w //= 2
                if g % 2 == 1:
                    b = g // 2
                    e_strip = ep.tile([128, F], _BF16, tag="e")
                    nc.vector.tensor_tensor(
                        out=e_strip[:, :].rearrange("p (a f) -> p a f", a=1),
                        in0=gs[g - 1][:, 0:1, :],
                        in1=gt[:, 0:1, :],
                        op=mybir.AluOpType.add,
                    )
                    nc.scalar.dma_start(
                        out=e_out[b * 128:(b + 1) * 128, :], in_=e_strip[:, :]
                    )
    nc.compile()
    return nc


def _build_kernel_b():
    nc = bacc.Bacc("TRN2", target_bir_lowering=True, num_swdge_queues=NQ,
                   dynamic_dma_scratch_size=65536)
    etab = nc.dram_tensor("etab", [EXT_ROWS, F], _BF16, kind="ExternalInput")
    # all constants packed into one [128, 520] int16 tensor:
    #   [0:192)   gather idxs (int16)
    #   [192:448) W1T as [128, 2, 128] bf16
    #   [448:512) W2T as [128, 64] bf16
    #   [512:516) b0 as [128, 2] f32
    #   [516:518) b1 as [128, 1] f32
    #   [518:520) b2 as [128, 1] f32 (partitions 0-63 live)
    CW = 520
    consts = nc.dram_tensor("consts", [128, CW], _I16, kind="ExternalInput")
    y = nc.dram_tensor("y", [1, SAMPLES_PER_CORE], _F32, kind="ExternalOutput")

    S = SAMPLES_PER_CORE                      # 1024
    L = 2 * S                                 # 2048 legs
    NI_W = NI_B // NWAVE                      # 1536 idxs per gather wave
    TN = 512                                  # matmul N-tile (one leg block)
    with TileContext(nc) as tc:
        nc.gpsimd.load_library(library_config.mlp)
        with (
            tc.tile_pool(name="const", bufs=1) as cp,
            tc.tile_pool(name="act", bufs=1) as ap,
            tc.tile_pool(name="ps", bufs=2, space="PSUM") as pp,
        ):
            ct = cp.tile([128, CW], _I16)
            nc.sync.dma_start(out=ct[:, :], in_=consts[:, :])
            idx_t = ct[:, 0:192]
            w1_t = ct[:, 192:448].bitcast(_BF16).rearrange(
                "p (c h) -> p c h", c=2)
            w2_t = ct[:, 448:512].bitcast(_BF16)
            b0_t = ct[:, 512:516].bitcast(_F32)
            b1_t = ct[:, 516:518].bitcast(_F32)
            b2_t = ct[:, 518:520].bitcast(_F32)
            ones = cp.tile([64, 1], _F32, tag="ones")
            nc.vector.memset(ones[:, :], 1.0)

            # fused gather, feature-major via transpose mode; wave w covers
            # samples [w*512, (w+1)*512) with columns blocked as
            # [E[d0] x512 | E[d1] x512 | cell x512] so every downstream
            # DVE op is unit-stride. One gather per 512-col block (large
            # transpose gathers hang on HW; 384-512 idxs is proven).
            xts = []
            gi = 0
            for w in range(NWAVE):
                wb = []
                for blk in range(3):
                    xt = ap.tile([128, 2, SW], _BF16, tag=f"xt{w}_{blk}")
                    nc.gpsimd.dma_gather(
                        xt[:, :, :], etab[:],
                        idx_t[:, (w * NI_W + blk * SW) // 16:
                              (w * NI_W + (blk + 1) * SW) // 16],
                        SW, SW, F,
                        # transpose-mode gathers corrupt (drop a 16-idx slot)
                        # with single_packet=False; they require True
                        transpose=True, single_packet=True, queue_num=gi % NQ,
                    )
                    gi += 1
                    wb.append(xt)
                xts.append(wb)

            # h0 column layout: [leg0_w0 | leg1_w0 | leg0_w1 | leg1_w1]
            pre = ap.tile([128, 2, L], _BF16, tag="pre")
            h0 = ap.tile([128, 2, L], _BF16, tag="h0")
            h1 = ap.tile([128, L], _BF16, tag="h1")
            h2 = ap.tile([64, L], _F32, tag="h2")
            prod = ap.tile([64, S], _F32, tag="prod")
            out_sb = ap.tile([1, S], _F32, tag="out")

            for w in range(NWAVE):
                for leg in range(2):
                    nc.vector.tensor_tensor(
                        out=pre[:, :, (2 * w + leg) * SW:(2 * w + leg + 1) * SW],
                        in0=xts[w][leg][:, :, :],
                        in1=xts[w][2][:, :, :],
                        op=mybir.AluOpType.add,
                    )
                # per leg-block tile: relu -> W1 -> relu -> W2 -> +b2
                for leg in range(2):
                    nt = 2 * w + leg
                    for c in range(2):
                        nc.scalar.activation(
                            h0[:, c, nt * TN:(nt + 1) * TN],
                            pre[:, c, nt * TN:(nt + 1) * TN],
                            mybir.ActivationFunctionType.Relu,
                            bias=b0_t[:, c:c + 1], scale=1.0,
                        )
                    ps1 = pp.tile([128, TN], _F32, tag="ps1")
                    for c in range(2):
                        nc.tensor.matmul(
                            ps1[:, :], w1_t[:, c, :],
                            h0[:, c, nt * TN:(nt + 1) * TN],
                            start=(c == 0), stop=(c == 1),
                        )
                    nc.scalar.activation(
                        h1[:, nt * TN:(nt + 1) * TN], ps1[:, :],
                        mybir.ActivationFunctionType.Relu,
                        bias=b1_t[:, 0:1], scale=1.0,
                    )
                    ps2 = pp.tile([64, TN], _F32, tag="ps2")
                    nc.tensor.matmul(
                        ps2[:, :], w2_t[:, :], h1[:, nt * TN:(nt + 1) * TN],
                        start=True, stop=True,
                    )
                    nc.scalar.activation(
                        h2[:, nt * TN:(nt + 1) * TN], ps2[:, :],
                        mybir.ActivationFunctionType.Identity,
                        bias=b2_t[0:64, 0:1], scale=1.0,
                    )
                # pair product for this wave's 512 samples
                nc.vector.tensor_tensor(
                    out=prod[:, w * SW:(w + 1) * SW],
                    in0=h2[:, 2 * w * TN:(2 * w + 1) * TN],
                    in1=h2[:, (2 * w + 1) * TN:(2 * w + 2) * TN],
                    op=mybir.AluOpType.mult,
                )
                ps3 = pp.tile([1, SW], _F32, tag="ps3")
                nc.tensor.matmul(
                    ps3[:, :], ones[:, :], prod[:, w * SW:(w + 1) * SW],
                    start=True, stop=True,
                )
                nc.vector.tensor_copy(
                    out_sb[:, w * SW:(w + 1) * SW], ps3[:, :]
                )
            nc.sync.dma_start(out=y[:, :], in_=out_sb[:, :])
    nc.compile()
    return nc


def _get_kernels():
    if "a" not in _cache:
        _cache["a"] = _build_kernel_a()
    if "b" not in _cache:
        _cache["b"] = _build_kernel_b()
    return _cache["a"], _cache["b"]


def _prep(drug_pairs, cell_lines, drug_targets, W0, b0, W1, b1, W2, b2):
    """Host-side data layout: shard, transpose, cast, build gather indices."""
    dt = np.asarray(drug_targets, dtype=np.int64)                  # [D, T]
    # dedup per row (reference uses .set -> dup targets count once)
    dup = (dt[:, :, None] == dt[:, None, :]) & (
        np.arange(T)[None, :, None] > np.arange(T)[None, None, :]
    )
    idx = np.where(dup.any(-1), ZROW, dt).astype(np.int32)          # [D, T]

    # W0T table: [P+C rows, F] bf16 + zero row + pad
    w0t = np.zeros((TAB_ROWS, F), dtype=ml_dtypes.bfloat16)
    w0t[: P + C] = np.asarray(W0, np.float32).T.astype(ml_dtypes.bfloat16)

    # launch A per-core gather index arrays
    idx_a = []
    for c in range(NCORES):
        rows = np.full((DRUGS_PAD, T), ZROW, np.int32)
        rows[:DRUGS_PER_CORE] = idx[c * DRUGS_PER_CORE:(c + 1) * DRUGS_PER_CORE]
        # flat j = b*4096 + t*128 + p  ->  drug 128b+p, target t
        flat = rows.reshape(4, 128, T).transpose(0, 2, 1).reshape(-1)
        idx_a.append(_wrap_idx(flat))

    # launch B per-core index arrays (built against E_ext layout), wave-
    # blocked: per wave of 512 samples, [e0 x512 | e1 x512 | cell x512]
    dp = np.asarray(drug_pairs, dtype=np.int64)                     # [B, 2]
    cl = np.asarray(cell_lines, dtype=np.int64)                     # [B]
    e_row = (dp // DRUGS_PER_CORE) * DRUGS_PAD + (dp % DRUGS_PER_CORE)
    cell_row = E_ROWS + cl
    idx_b = []
    for c in range(NCORES):
        blocks = []
        for w in range(NWAVE):
            sl = slice(c * SAMPLES_PER_CORE + w * SW,
                       c * SAMPLES_PER_CORE + (w + 1) * SW)
            blocks += [e_row[sl, 0], e_row[sl, 1], cell_row[sl]]
        idx_b.append(_wrap_idx(np.concatenate(blocks)))

    # packed launch-B constants [128, 520] int16 (see _build_kernel_b)
    w1t = np.asarray(W1, np.float32).T.astype(ml_dtypes.bfloat16)   # [F, H1]
    w2t = np.asarray(W2, np.float32).T.astype(ml_dtypes.bfloat16)   # [H1, H2]
    b0c = np.asarray(b0, np.float32).reshape(2, 128).T.copy()       # [128, 2]
    b1c = np.asarray(b1, np.float32).reshape(128, 1)
    b2c = np.zeros((128, 1), np.float32)
    b2c[:64] = np.asarray(b2, np.float32).reshape(64, 1)
    consts = []
    for c in range(NCORES):
        buf = np.zeros((128, 520), np.int16)
        buf[:, 0:192] = idx_b[c]
        buf[:, 192:448] = w1t.reshape(2, 128, H1).transpose(1, 0, 2).reshape(
            128, 256).view(np.int16)
        buf[:, 448:512] = w2t.view(np.int16)
        buf[:, 512:516] = b0c.view(np.int16)
        buf[:, 516:518] = b1c.view(np.int16)
        buf[:, 518:520] = b2c.view(np.int16)
        consts.append(buf)

    celltab = np.asarray(W0, np.float32)[:, P:P + C].T.astype(
        ml_dtypes.bfloat16)                                         # [C, F]
    return w0t, idx_a, consts, celltab


def _run(inputs, trace=False):
    nca, ncb = _get_kernels()
    w0t, idx_a, consts, celltab = _prep(**inputs)

    in_a = [{"tab": w0t, "idxs": idx_a[c]} for c in range(NCORES)]
    res_a = run_bass_kernel_spmd(
        nca, in_a, core_ids=list(range(NCORES)), trace=trace)

    e_ext = np.concatenate(
        [res_a.results[c]["e_out"] for c in range(NCORES)] + [celltab], axis=0
    )
    assert e_ext.shape == (EXT_ROWS, F)

    in_b = [{"etab": e_ext, "consts": consts[c]} for c in range(NCORES)]
    res_b = run_bass_kernel_spmd(
        ncb, in_b, core_ids=list(range(NCORES)), trace=trace)

    out = np.concatenate(
        [res_b.results[c]["y"].reshape(-1) for c in range(NCORES)]
    ).astype(np.float32)
    times = (res_a.exec_time_ns, res_b.exec_time_ns)
    return out, times


def kernel(**inputs) -> np.ndarray:
    out, _ = _run(inputs, trace=False)
    return out
